# revision 7
# baseline (speedup 1.0000x reference)
"""Trainium2 Bass kernel v2 for nn_BinaryGroupConv.

Reference op (per image): BatchNorm2d (inference) -> sign-binarize ->
grouped 3x3 conv (64 groups, 4->4 ch, binarized weights) -> channel
shuffle -> residual add.

v2 strategy (vs the bf16 baseline):
  - x is read from HBM ONCE (25.7 MB/core total traffic vs 38.5): the
    single per-image load places channel 64i+32c+q on partition 32i+q of
    block c ("xr" layout = shuffled-residual order). The residual add then
    reads xr directly from SBUF, partition-aligned with the psum output
    order m = 32i+q.
  - Conv runs in fp8 (e4m3; +-1 and 0 exact) with DoubleRow perf mode:
    two taps share one matmul pass via a pair-plane rhs AP whose plane
    stride is the tap-offset delta in the zero-padded 58x58 activation
    buffer (overlapping planes; AP dim mutated in place). 9 taps ->
    4 DoubleRow + 1 single pass: ~1.5x less PE time than 9 bf16 passes.
  - BN affine (x*inv, rounded) runs 64-wide on GPSIMD/DVE into a K-ordered
    scratch; ACT applies Sign(scratch + t) 128-wide into fp8. Separate
    rounding of mul and add matches the eager jax reference bit-for-bit.
  - The psum drain IS the residual add: one 128-wide DVE tensor_tensor per
    tile (ps + xr -> dense staging), then one 1.6 MB store per chunk with
    the shuffle folded into the dram access pattern.
"""

import numpy as np

import jax
import ml_dtypes

import concourse.bacc as bacc
import concourse.tile as tile
from concourse import mybir
from contextlib import ExitStack

N_CORES = 8
IMG = 4           # images per core
C = 256
H = W = 56
HW = H * W
HP = 58           # padded row length
GRID = HP * HP    # 58x58 padded image
APAD = GRID + 2   # +1 guard element on each side
ROWS = 8          # output rows per tile
NT = H // ROWS    # 7 tiles per chunk
TN = ROWS * HP    # 464 matmul free dim (padded mode)
EPS = 1e-5

# DoubleRow tap pairing: offsets s(dh, dw) = 1 + HP*(8t + 1 + dh) + dw.
PAIRS = [((-1, -1), (-1, 0)), ((0, -1), (0, 0)), ((1, -1), (1, 0)),
         ((-1, 1), (0, 1))]
SINGLE = (1, 1)
WORDER = [t for p in PAIRS for t in p] + [SINGLE]  # weight buffer tap order

AFFINE_ENGINES = ("vector", "vector")

_CACHE = {}


def _ch(m, b):
    """x channel held at xr partition m, block b."""
    return 64 * (m // 32) + 32 * b + (m % 32)


def _kch(c, k):
    """x channel at K-row k of conv chunk c's activation buffer."""
    if c == 0:
        return _ch(k, 0) if k < 64 else _ch(k - 64, 1)
    return _ch(64 + k, 0) if k < 64 else _ch(k, 1)


def _build_program(repeat=1, stages=frozenset({'affine','sign','mm','adds','loads','stores'})):
    nc = bacc.Bacc("TRN2")
    f32 = mybir.dt.float32
    fp8 = mybir.dt.float8e4
    x_in = nc.declare_dram_parameter("x", [IMG, C, H, W], f32, isOutput=False)
    wt_in = nc.declare_dram_parameter("wt", [128, 18 * 128], fp8, isOutput=False)
    bn_in = nc.declare_dram_parameter("bn", [128, 4], f32, isOutput=False)
    y_out = nc.declare_dram_parameter("y", [IMG, C, H, W], f32, isOutput=True)

    with tile.TileContext(nc) as tc, ExitStack() as ctx:
        const_pool = ctx.enter_context(tc.tile_pool(name="const", bufs=1))
        apad_pool = ctx.enter_context(tc.tile_pool(name="apad", bufs=1))
        xr_pool = ctx.enter_context(tc.tile_pool(name="xr", bufs=4))
        sc_pool = ctx.enter_context(tc.tile_pool(name="sc", bufs=1))
        ys_pool = ctx.enter_context(tc.tile_pool(name="ys", bufs=4))
        psum_pool = ctx.enter_context(
            tc.tile_pool(name="ps", bufs=7, space="PSUM")
        )

        # Trigger the ACT table load (Sign set, ~2.7us) immediately so it
        # overlaps the first DMAs instead of the first real activation.
        warm = const_pool.tile([128, 2], f32, tag="actwarm")
        nc.vector.memset(warm[:], 0.0)
        nc.scalar.activation(warm[:], warm[:], mybir.ActivationFunctionType.Sign)

        bn_sb = const_pool.tile([128, 4], f32, tag="bn")
        nc.sync.dma_start(bn_sb[:], bn_in[:])
        wt_sb = const_pool.tile([128, 18 * 128], fp8, tag="wt")
        nc.sync.dma_start(wt_sb[:], wt_in[:])

        # fp8 activation buffers, zero-padded 58x58 layout (+ tail pad so
        # overlapping DoubleRow pair views stay in bounds). Pads are zeroed
        # once; the interior is rewritten by ACT before every use.
        apads = []
        for b in range(3):
            ap_t = apad_pool.tile([128, APAD + 2 * TN], fp8, tag=f"apad{b}")
            nc.vector.memset(ap_t[:, 0:59], 0.0)  # guard + top pad row
            nc.vector.memset(  # right pad of row r | left pad of row r+1
                ap_t[:, 58 : 58 + 57 * HP].rearrange("p (r z) -> p r z", z=HP)[
                    :, :, 0:2
                ],
                0.0,
            )
            nc.vector.memset(ap_t[:, 1 + 57 * HP : APAD + 2 * TN], 0.0)
            apads.append(ap_t)
        # Padded f32 scratch (x*inv + t), pads zeroed once; Sign then runs
        # dense (sign(0) = 0 keeps the pads zero in the fp8 buffer).
        scs = []
        for b in range(2):
            sc_t = sc_pool.tile([128, APAD], f32, tag=f"sc{b}")
            nc.vector.memset(sc_t[:], 0.0)
            scs.append(sc_t)

        chunks = [
            (img, c)
            for _rep in range(repeat)
            for img in range(IMG)
            for c in range(2)
        ]
        xrs = {}

        def load_xr(img, first=False):
            xr = xr_pool.tile([128, 2, HW], f32, tag="xr")
            # partition m = 32i+q, block b <- channel 64i+32b+q. One DMA per
            # contiguous 32-channel run: a split-partition dram AP in a
            # single DMA measures 3x slower than these 8 plain transfers.
            # The very first load is split into row-halves so the first
            # chunk's affine/sign can start at the half-load mark.
            row_splits = ((0, 28), (28, H)) if first else ((0, H),)
            for r0, r1 in row_splits:
                for b in range(2):
                    for i in range(4):
                        ch0 = 64 * i + 32 * b
                        nc.sync.dma_start(
                            xr[32 * i : 32 * i + 32, b, r0 * W : r1 * W],
                            x_in[img, ch0 : ch0 + 32, r0:r1].rearrange(
                                "c h w -> c (h w)"
                            ),
                        )
            return xr

        signed = [None] * len(chunks)
        for k, (img, c) in enumerate(chunks):
            signed[k] = _emit_prep(nc, k, chunks, xrs, bn_sb, apads,
                                   scs, load_xr, stages)
            if k >= 1:
                _emit_compute(nc, signed[k - 1], y_out, wt_sb, psum_pool,
                              ys_pool, stages)
        _emit_compute(nc, signed[-1], y_out, wt_sb, psum_pool, ys_pool,
                      stages)
    nc.compile()
    return nc


def _emit_prep(nc, k, chunks, xrs, bn_sb, apads, scs, load_xr, stages):
    img, c = chunks[k]
    f32 = mybir.dt.float32
    rep = k // 8
    if 'loads' not in stages:
        if (img, rep) not in xrs:
            xrs[(img, rep)] = xrs.get('static') or load_xr(img)
            xrs['static'] = xrs[(img, rep)]
    else:
        if (img, rep) not in xrs:
            xrs[(img, rep)] = load_xr(img, first=(k == 0))
        # prefetch the next two images' xr early (loads must clear the
        # DMA FIFO before their image's prep starts)
        nk = k + 2 - (k % 2)  # start of the next image's chunks
        for nk_i in (nk, nk + 2):
            if nk_i < len(chunks):
                nimg, nrep = chunks[nk_i][0], nk_i // 8
                if (nimg, nrep) not in xrs:
                    xrs[(nimg, nrep)] = load_xr(nimg)
    xr = xrs[(img, rep)]
    ap_t = apads[k % 3]
    _maybe_affine_sign(nc, c, xr, scs[c], bn_sb, ap_t, stages)
    return (img, c, ap_t, xr)


def _maybe_affine_sign(nc, c, xr, sc, bn_sb, ap_t, stages):
    # BN affine per row-piece: sc[k-row] = (x * inv) + t with both stages
    # rounded separately (DVE two-op tensor_scalar), written into the
    # zero-padded scratch; then Sign runs DENSE (in and out contiguous:
    # sign(0) = 0 keeps all pad cells zero in the fp8 buffer).
    half = slice(0, 64) if c == 0 else slice(64, 128)
    for hh in range(NT):
        r0 = hh * ROWS
        off = 1 + HP * (r0 + 1)
        if 'affine' in stages:
            sc_rows = sc[:, off + 1 : off + 1 + ROWS * HP].rearrange(
                "p (h w) -> p h w", w=HP
            )[:, :, 0:W]
            for lo, hi, b in ((0, 64, 0), (64, 128, 1)):
                nc.vector.tensor_scalar(
                    sc_rows[lo:hi],
                    xr[half, b, r0 * W : (r0 + ROWS) * W].rearrange(
                        "p (h w) -> p h w", w=W
                    ),
                    bn_sb[lo:hi, 2 * c : 2 * c + 1],
                    bn_sb[lo:hi, 2 * c + 1 : 2 * c + 2],
                    op0=mybir.AluOpType.mult,
                    op1=mybir.AluOpType.add,
                )
        if 'sign' in stages:
            nc.scalar.activation(
                ap_t[:, off : off + ROWS * HP],
                sc[:, off : off + ROWS * HP],
                mybir.ActivationFunctionType.Sign,
            )


def _emit_compute(nc, stage, y_out, wt_sb, psum_pool, ys_pool, stages):
    img, c, ap_t, xr = stage
    f32 = mybir.dt.float32
    ys = ys_pool.tile([128, HW], f32, tag="ys")
    if 'mm' not in stages and 'adds' not in stages:
        nc.vector.memset(ys[:, 0:4], 0.0)
    for t in range(NT):
        if 'mm' not in stages and 'adds' not in stages:
            break
        ps = psum_pool.tile([128, TN], f32, tag="ps")
        if 'mm' not in stages:
            if 'adds' in stages and t == 0:
                nc.vector.memset(ps[:], 0.0)
            if 'adds' in stages:
                _emit_add(nc, ys, ps, xr, c, t)
            continue
        for i, (ta, tb) in enumerate(PAIRS):
            sa = 1 + HP * (ROWS * t + 1 + ta[0]) + ta[1]
            sb = 1 + HP * (ROWS * t + 1 + tb[0]) + tb[1]
            rhs = ap_t[:, sa : sa + 2 * TN].rearrange(
                "p (two n) -> p two n", two=2
            )
            rhs.ap[1] = [sb - sa, 2]
            lhsT = wt_sb[
                :, (9 * c + 2 * i) * 128 : (9 * c + 2 * i + 2) * 128
            ].rearrange("p (two m) -> p two m", two=2)
            nc.tensor.matmul(
                ps[:], lhsT, rhs,
                start=(i == 0), stop=False,
                perf_mode=mybir.MatmulPerfMode.DoubleRow,
            )
        s = 1 + HP * (ROWS * t + 1 + SINGLE[0]) + SINGLE[1]
        nc.tensor.matmul(
            ps[:], wt_sb[:, (9 * c + 8) * 128 : (9 * c + 9) * 128],
            ap_t[:, s : s + TN],
            start=False, stop=True,
        )
        if 'adds' in stages:
            _emit_add(nc, ys, ps, xr, c, t)
        elif t == NT - 1:
            nc.vector.tensor_copy(ys[:, 0:TN], ps[:])
    # One store per chunk; the shuffle is folded into the dram AP
    # (partition 32i+q -> channel 64i+32c+q, contiguous 12.5 KB runs).
    if 'stores' in stages:
        # Second HWDGE ring (ACT sequencer): store bursts never queue ahead
        # of prefetched loads in the SP ring's FIFO.
        for i in range(4):
            ch0 = 64 * i + 32 * c
            nc.scalar.dma_start(
                y_out[img, ch0 : ch0 + 32].rearrange("c h w -> c (h w)"),
                ys[32 * i : 32 * i + 32, :],
            )


def _emit_add(nc, ys, ps, xr, c, t):
    # Drain + residual in one op: ys = ps(valid cols) + xr (exact f32).
    nc.vector.tensor_tensor(
        ys[:, t * ROWS * W : (t + 1) * ROWS * W].rearrange(
            "p (h w) -> p h w", w=W
        ),
        ps[:].rearrange("p (h w) -> p h w", w=HP)[:, :, 1 : 1 + W],
        xr[:, c, t * ROWS * W : (t + 1) * ROWS * W].rearrange(
            "p (h w) -> p h w", w=W
        ),
        op=mybir.AluOpType.add,
    )


def _pack_weights(weight):
    """fp8 per-tap lhsT tiles in WORDER with shuffle-folded output order.

    wt[k, (9c+j)*128 + m]: psum partition m = 32i+q holds conv output
    channel oc = 128c + 4q + i; K-row k holds input channel _kch(c, k).
    Nonzero iff _kch(c,k) is in group 32c+q, value sign(weight[...]).
    """
    ws = np.sign(weight.astype(np.float32))  # [256, 4, 3, 3]
    wt = np.zeros((128, 2, 9, 128), np.float32)
    for c in range(2):
        kch = np.array([_kch(c, k) for k in range(128)])  # K-row -> channel
        m = np.arange(128)
        oc = 128 * c + 4 * (m % 32) + m // 32
        g = oc // 4  # group of each output (= 32c + q)
        for j, (dh, dw) in enumerate(WORDER):
            kh, kw = dh + 1, dw + 1
            # wt[k, m] = ws[oc[m], kch[k] - 4*g[m], kh, kw] if in group
            kk, mm = np.meshgrid(kch, oc, indexing="ij")
            ingrp = (kk // 4) == (mm // 4)
            idx = np.clip(kk - 4 * (mm // 4), 0, 3)
            wt[:, c, j, :] = np.where(ingrp, ws[mm, idx, kh, kw], 0.0)
    return wt.reshape(128, 18 * 128).astype(ml_dtypes.float8_e4m3)


def _pack_bn(gamma, beta, running_mean, running_var):
    # Mirror the reference ops (and platform) bit-for-bit.
    import jax.numpy as jnp

    inv = np.asarray(
        jnp.asarray(gamma) * jax.lax.rsqrt(jnp.asarray(running_var) + EPS)
    )
    t = np.asarray(
        jnp.asarray(beta) - jnp.asarray(running_mean) * jnp.asarray(inv)
    )
    bn = np.zeros((128, 4), np.float32)
    for c in range(2):
        kch = np.array([_kch(c, k) for k in range(128)])
        bn[:, 2 * c] = inv[kch]
        bn[:, 2 * c + 1] = t[kch]
    return bn


def _get_runner():
    if "runner" in _CACHE:
        return _CACHE["runner"]
    runner = _make_runner(_build_program())
    _CACHE["runner"] = runner
    return runner


def _make_runner(nc):
    from jax.sharding import Mesh, PartitionSpec, NamedSharding
    from jax.experimental.shard_map import shard_map
    from concourse import bass2jax

    bass2jax.install_neuronx_cc_hook()

    partition_name = (
        nc.partition_id_tensor.name if nc.partition_id_tensor is not None else None
    )
    in_names = []
    out_names = []
    out_avals = []
    for alloc in nc.m.functions[0].allocations:
        if not isinstance(alloc, mybir.MemoryLocationSet):
            continue
        name = alloc.memorylocations[0].name
        if alloc.kind == "ExternalInput":
            if name != partition_name:
                in_names.append(name)
        elif alloc.kind == "ExternalOutput":
            out_names.append(name)
            out_avals.append(
                jax.core.ShapedArray(
                    tuple(alloc.tensor_shape), mybir.dt.np(alloc.dtype)
                )
            )
    n_params = len(in_names)
    bind_in_names = tuple(
        in_names + out_names + ([partition_name] if partition_name else [])
    )

    def _body(*args):
        operands = list(args)
        if partition_name is not None:
            operands.append(bass2jax.partition_id_tensor())
        outs = bass2jax._bass_exec_p.bind(
            *operands,
            out_avals=tuple(out_avals),
            in_names=bind_in_names,
            out_names=tuple(out_names),
            lowering_input_output_aliases=(),
            sim_require_finite=True,
            sim_require_nnan=True,
            nc=nc,
        )
        return tuple(outs)

    devices = jax.devices()[:N_CORES]
    mesh = Mesh(np.asarray(devices), ("core",))
    spec = PartitionSpec("core")
    n_out = len(out_names)
    sharded = jax.jit(
        shard_map(
            _body,
            mesh=mesh,
            in_specs=(spec,) * (n_params + n_out),
            out_specs=(spec,) * n_out,
            check_rep=False,
        ),
        keep_unused=True,
    )
    sharding = NamedSharding(mesh, spec)
    zeros = [
        jax.device_put(
            np.zeros((N_CORES * a.shape[0], *a.shape[1:]), a.dtype), sharding
        )
        for a in out_avals
    ]
    return dict(
        nc=nc,
        fn=sharded,
        in_names=in_names,
        out_names=out_names,
        sharding=sharding,
        zeros=zeros,
    )


def _device_inputs(x, weight, gamma, beta, running_mean, running_var):
    """Host-side packing -> concatenated per-core arrays on the 8 devices."""
    r = _get_runner()
    wt = np.asarray(_pack_weights(np.asarray(weight, np.float32)))
    bn = _pack_bn(
        np.asarray(gamma, np.float32),
        np.asarray(beta, np.float32),
        np.asarray(running_mean, np.float32),
        np.asarray(running_var, np.float32),
    )
    x = np.ascontiguousarray(np.asarray(x, np.float32))
    concat = {
        "x": x.reshape(N_CORES * IMG, C, H, W),
        "wt": np.concatenate([wt] * N_CORES, axis=0),
        "bn": np.concatenate([bn] * N_CORES, axis=0),
    }
    args = [
        jax.device_put(concat[name], r["sharding"]) for name in r["in_names"]
    ]
    return r, args


def kernel(x, weight, gamma, beta, running_mean, running_var):
    r, args = _device_inputs(x, weight, gamma, beta, running_mean, running_var)
    outs = r["fn"](*args, *r["zeros"])
    y = np.asarray(outs[0])
    return y.reshape(N_CORES * IMG, C, H, W)
